# revision 30
# baseline (speedup 1.0000x reference)
"""Trainium2 Bass kernel: ColumnParallelLinear + multi-adapter LoRA routing.

Computes out = x @ W^T + bias + B[aid[s]] @ (A[aid[s]] @ x[s]) for each token.

Distribution across 8 NeuronCores (one TRN2 chip):
  - base GEMM is tensor-parallel over d_out (sharding_hint): weight + bias
    sharded, each core emits out_base^T [512, S]
  - the LoRA delta is token-parallel: core i computes the delta for ITS
    1024-token slab across ALL d_out (A and B are tiny and replicated); the
    host adds the two partial results while unsharding
  - each core's token axis is ROTATED on the host so its own slab occupies
    the first two 512-token tiles; the xa matmuls then reuse the base
    x-strips already in SBUF; the host un-rotates while unsharding

Precision strategy (rel-err budget 2e-2, measured 1.95e-2 end to end):
  - the first N8=12 of 32 k-tiles run in fp8 e4m3 with perf_mode=DoubleRow:
    one pair-instruction contracts K=256 in the time of a single bf16 MM
    (measured 216ns back-to-back, the full 2x) -> 6 pair MMs replace 12
  - fp8 operands are pre-scaled on the host (x*16, w*1024); the remaining
    bf16 k-tiles use weights pre-scaled by C=16384 so both parts accumulate
    in the SAME PSUM group; outputs are stored C-scaled in bf16 and the
    host divides while unsharding (no extra on-chip work)
  - the LoRA-A projection uses the same k-split; B-matmuls stay bf16

Schedule:
  - startup: fp8 w + fp8 x-strip0 land first (1.5MB in a few large DMAs at
    ~400GB/s aggregate - single dma_starts stripe over all 16 DMA engines);
    j=0 runs "k-outer" so every arriving k-chunk immediately feeds all 4
    m-tile PSUM groups -> PE is busy ~2us after the DMA preamble
  - bias is added during base PSUM->SBUF eviction (per-partition scalar add)
  - the 64 B_cat delta matmuls are drip-fed 2-per-base-evict
  - outputs are stored as scaled bf16 (halves store traffic; the final
    stores are split into pieces so the kernel tail stays short)
"""

import os
import sys

import numpy as np

try:
    import ml_dtypes
except ImportError:  # pragma: no cover
    sys.path.insert(0, "/opt/trn_rl_repo")
    import ml_dtypes

_P = 128  # SBUF partitions / matmul tile edge
_NT = 512  # token tile (matmul moving free dim, one PSUM bank of fp32)
_LR = 128  # L * R = 8 * 16 adapter-rank rows
_N_CORES = 8
_N8 = 12  # k-tiles (of 32) computed in fp8 e4m3 DoubleRow
_SX = np.float32(16.0)  # x fp8 scale
_SW = np.float32(1024.0)  # w fp8 scale
_SA = np.float32(1024.0)  # A fp8 scale

_NC_CACHE = {}
LAST_RESULTS = None  # BassKernelResults of the most recent run (for test.py)


def _import_concourse():
    try:
        import concourse  # noqa: F401
    except ImportError:  # pragma: no cover
        for p in ("/opt/trn_rl_repo", "/root/.axon_site/_ro/trn_rl_repo"):
            if os.path.isdir(p) and p not in sys.path:
                sys.path.insert(0, p)


def build_nc(d_in: int, d_loc: int, s_tokens: int, s_own: int, d_out: int):
    """Build + finalize the per-core Bass kernel.

    d_loc: output features of this core's base shard
    s_own: tokens in this core's LoRA-delta slab (the FIRST s_own tokens of
           the core's rotated token order)
    d_out: full output width (the delta covers all of it)
    """
    _import_concourse()
    import concourse.tile as tile
    from concourse import bacc, mybir

    P, NT, LR, N8 = _P, _NT, _LR, _N8
    n_kt = d_in // P
    NB = n_kt - N8  # bf16 k-tiles
    NPR = N8 // 2  # fp8 DoubleRow pairs
    n_mt = d_loc // P
    n_nt = s_tokens // NT
    n_ot = s_own // NT  # own-slab token tiles
    n_dt = d_out // P  # delta feature tiles
    assert all(v % P == 0 for v in (d_in, d_loc, d_out)) and s_tokens % NT == 0
    assert s_own % NT == 0 and n_ot <= n_nt and N8 % 2 == 0

    nc = bacc.Bacc("TRN2", target_bir_lowering=False, debug=False)

    bf16 = mybir.dt.bfloat16
    f8 = mybir.dt.float8e4
    f32 = mybir.dt.float32
    DR = mybir.MatmulPerfMode.DoubleRow

    x8T = nc.dram_tensor("x8T", [N8 * P, s_tokens], f8, kind="ExternalInput").ap()
    xbT = nc.dram_tensor("xbT", [NB * P, s_tokens], bf16, kind="ExternalInput").ap()
    w8_t = nc.dram_tensor("w8_t", [N8 * P, d_loc], f8, kind="ExternalInput").ap()
    wb_t = nc.dram_tensor("wb_t", [NB * P, d_loc], bf16, kind="ExternalInput").ap()
    a8_t = nc.dram_tensor("a8_t", [N8 * P, LR], f8, kind="ExternalInput").ap()
    ab_t = nc.dram_tensor("ab_t", [NB * P, LR], bf16, kind="ExternalInput").ap()
    b_cat_t = nc.dram_tensor("b_cat_t", [LR, d_out], bf16, kind="ExternalInput").ap()
    mask_own = nc.dram_tensor("mask_own", [LR, s_own], bf16, kind="ExternalInput").ap()
    bias_pre = nc.dram_tensor("bias_pre", [P, n_mt], f32, kind="ExternalInput").ap()
    out_t = nc.dram_tensor("out_t", [d_loc, s_tokens], bf16, kind="ExternalOutput").ap()
    delta_t = nc.dram_tensor("delta_t", [d_out, s_own], bf16, kind="ExternalOutput").ap()

    # [kt*128 + p, n] -> [p, kt, n]
    x8_v = x8T.rearrange("(kt p) s -> p kt s", p=P)
    xb_v = xbT.rearrange("(kt p) s -> p kt s", p=P)
    w8_v = w8_t.rearrange("(kt p) m -> p kt m", p=P)
    wb_v = wb_t.rearrange("(kt p) m -> p kt m", p=P)
    a8_v = a8_t.rearrange("(kt p) m -> p kt m", p=P)
    ab_v = ab_t.rearrange("(kt p) m -> p kt m", p=P)

    # startup chunking: fp8 pairs first in fine chunks (PE starts ~2us after
    # the preamble), then bf16 weight/strip chunks interleaved 4 k-tiles at
    # a time — the head is chip-bandwidth-bound with all 8 cores loading at
    # once, so arrival order must match the j=0 k-outer consumption order
    PAIR_CHUNKS = [(0, 1), (1, 2), (2, 4), (4, NPR)]
    BF_CHUNKS = [(c, min(c + 4, NB)) for c in range(0, NB, 4)]

    with tile.TileContext(nc) as tc:
        with (
            tc.tile_pool(name="const", bufs=1) as const_pool,
            tc.tile_pool(name="xp", bufs=1) as x_pool,
            tc.tile_pool(name="outp", bufs=1) as out_pool,
            tc.tile_pool(name="psum", bufs=1, space="PSUM") as psum_pool,
        ):
            w8_all = const_pool.tile([P, N8, d_loc], f8)
            wb_all = const_pool.tile([P, NB, d_loc], bf16)
            b_cat = const_pool.tile([P, d_out], bf16)
            bias_sb = const_pool.tile([P, n_mt], f32)
            a8_all = const_pool.tile([P, N8, LR], f8)
            ab_all = const_pool.tile([P, NB, LR], bf16)
            xa_sb = const_pool.tile([P, s_own], bf16)
            mask_sb = const_pool.tile([P, s_own], bf16)

            # All PSUM tiles share one 8-bank ring so that during the DMA-
            # paced startup TWO token-tiles' worth of fp8 groups (8 banks)
            # can be open at once: the fp8 DoubleRow work of j and j+1 runs
            # one iteration ahead of j's bf16 work (software pipelining),
            # which keeps the PE busy while the 5MB bf16 block streams in.
            def ring_tile(name):
                return psum_pool.tile([P, NT], f32, tag="ring", bufs=8, name=name)

            # Deferred LoRA-delta jobs, drip-fed between base m-tiles.
            delta_jobs = []

            def emit_delta(k):
                for _ in range(k):
                    if not delta_jobs:
                        return
                    n, m = delta_jobs.pop(0)
                    dl_ps = ring_tile(f"dl_ps{n}_{m}")
                    nc.tensor.matmul(
                        dl_ps[:],
                        b_cat[:, m * P : (m + 1) * P],
                        xa_sb[:, n * NT : (n + 1) * NT],
                        start=True,
                        stop=True,
                    )
                    d_sb = out_pool.tile(
                        [P, NT], bf16, tag="d_sb", bufs=4, name=f"d_sb{n}_{m}"
                    )
                    nc.scalar.copy(d_sb[:], dl_ps[:])
                    nc.sync.dma_start(
                        delta_t[m * P : (m + 1) * P, n * NT : (n + 1) * NT], d_sb[:]
                    )

            def load_x8_strip(j):
                tok = slice(j * NT, (j + 1) * NT)
                x8s = x_pool.tile([P, N8, NT], f8, tag="x8s", bufs=4, name=f"x8s{j}")
                nc.sync.dma_start(x8s[:], x8_v[:, :, tok])
                return x8s

            def load_xb_strip(j):
                tok = slice(j * NT, (j + 1) * NT)
                xbs = x_pool.tile([P, NB, NT], bf16, tag="xbs", bufs=4, name=f"xbs{j}")
                for c, e in ((0, NB // 2), (NB // 2, NB)):
                    nc.sync.dma_start(xbs[:, c:e, :], xb_v[:, c:e, tok])
                return xbs

            def evict_base(j, m, ps):
                tok0 = j * NT
                o_sb = out_pool.tile(
                    [P, NT], bf16, tag="o_sb", bufs=6, name=f"o_sb{j}_{m}"
                )
                nc.vector.tensor_scalar_add(
                    out=o_sb[:], in0=ps[:], scalar1=bias_sb[:, m : m + 1]
                )
                nc.sync.dma_start(
                    out_t[m * P : (m + 1) * P, tok0 : tok0 + NT], o_sb[:]
                )
                emit_delta(2)

            def base_group_mms(ps, m, x8s, xbs, pr_range, kb_range):
                for pr in pr_range:
                    nc.tensor.matmul(
                        ps[:],
                        w8_all[:, 2 * pr : 2 * pr + 2, m * P : (m + 1) * P],
                        x8s[:, 2 * pr : 2 * pr + 2, :],
                        start=(pr == 0),
                        stop=False,
                        perf_mode=DR,
                    )
                for kt in kb_range:
                    nc.tensor.matmul(
                        ps[:],
                        wb_all[:, kt, m * P : (m + 1) * P],
                        xbs[:, kt, :],
                        start=False,
                        stop=(kt == NB - 1),
                    )

            pss = {}  # j -> list of 4 open PSUM groups (fp8 part done)

            def alloc_groups(j):
                pss[j] = [ring_tile(f"ps{j}_{m}") for m in range(n_mt)]

            def dr_ntile(j, x8s, pair_chunks):
                # the fp8 DoubleRow half of token-tile j, k-outer across all
                # m-tiles per pair chunk
                for c, e in pair_chunks:
                    for m in range(n_mt):
                        base_group_mms(pss[j][m], m, x8s, None, range(c, e), ())

            def bf_half_group(j, m, xbs, h):
                # last group of the kernel: run the bf16 part as two N=256
                # halves so the first half's evict+store overlaps the second
                # half's matmuls and the kernel tail shrinks
                ps = pss[j][m]
                sl = slice(h * (NT // 2), (h + 1) * (NT // 2))
                for kt in range(NB):
                    nc.tensor.matmul(
                        ps[:, sl],
                        wb_all[:, kt, m * P : (m + 1) * P],
                        xbs[:, kt, sl],
                        start=False,
                        stop=(kt == NB - 1),
                        skip_group_check=True,
                    )
                o_sb = out_pool.tile(
                    [P, NT // 2], bf16, tag="o_half", bufs=2, name=f"oh{j}_{m}_{h}"
                )
                nc.vector.tensor_scalar_add(
                    out=o_sb[:], in0=ps[:, sl], scalar1=bias_sb[:, m : m + 1]
                )
                nc.sync.dma_start(
                    out_t[
                        m * P : (m + 1) * P,
                        j * NT + h * (NT // 2) : j * NT + (h + 1) * (NT // 2),
                    ],
                    o_sb[:],
                )

            def bf_ntile(j, xbs, k_outer=False):
                # the bf16 half; for j=0 consume 4-k-tile chunks across all
                # m-tiles in DMA arrival order
                if not k_outer:
                    for m in range(n_mt):
                        if j == n_nt - 1 and m == n_mt - 1:
                            bf_half_group(j, m, xbs, 0)
                            bf_half_group(j, m, xbs, 1)
                            emit_delta(2)
                        else:
                            base_group_mms(pss[j][m], m, None, xbs, (), range(NB))
                            evict_base(j, m, pss[j][m])
                    return
                for c, e in BF_CHUNKS:
                    for m in range(n_mt):
                        base_group_mms(pss[j][m], m, None, xbs, (), range(c, e))
                for m in range(n_mt):
                    evict_base(j, m, pss[j][m])

            def xa_block(n, x8s, xbs):
                # xa = A_all @ x^T for own-slab tile n, masked per-token;
                # queues that tile's 32 B_cat delta matmuls
                xa_ps = ring_tile(f"xa_ps{n}")
                for pr in range(NPR):
                    nc.tensor.matmul(
                        xa_ps[:],
                        a8_all[:, 2 * pr : 2 * pr + 2, :],
                        x8s[:, 2 * pr : 2 * pr + 2, :],
                        start=(pr == 0),
                        stop=False,
                        perf_mode=DR,
                    )
                for kt in range(NB):
                    nc.tensor.matmul(
                        xa_ps[:],
                        ab_all[:, kt, :],
                        xbs[:, kt, :],
                        start=False,
                        stop=(kt == NB - 1),
                    )
                nc.vector.tensor_mul(
                    out=xa_sb[:, n * NT : (n + 1) * NT],
                    in0=xa_ps[:],
                    in1=mask_sb[:, n * NT : (n + 1) * NT],
                )
                delta_jobs.extend((n, m) for m in range(n_dt))

            # ---- startup DMA: all on the Sync queue (transfers issued from
            # the Scalar/GpSimd queues measured slower), in consumption
            # order: fp8 pairs for j=0, bias, the j=1 fp8 strip (feeds the
            # pipelined-ahead DR work), interleaved bf16 w/x chunks, the
            # rest of strips j=1,2, LoRA constants
            x8s0 = x_pool.tile([P, N8, NT], f8, tag="x8s", bufs=4, name="x8s_first")
            xbs0 = x_pool.tile([P, NB, NT], bf16, tag="xbs", bufs=4, name="xbs_first")
            x8strips = {0: x8s0}
            xbstrips = {0: xbs0}
            for ci, (c, e) in enumerate(PAIR_CHUNKS):
                nc.sync.dma_start(
                    w8_all[:, 2 * c : 2 * e, :], w8_v[:, 2 * c : 2 * e, :]
                )
                nc.sync.dma_start(
                    x8s0[:, 2 * c : 2 * e, :], x8_v[:, 2 * c : 2 * e, 0:NT]
                )
                if ci == 1 and n_nt > 1:
                    # j=1's fp8 strip feeds the pipelined-ahead DR work: land
                    # it inside j=0's fp8 block, once the PE has a 2-pair
                    # backlog to chew on
                    x8strips[1] = load_x8_strip(1)
            nc.sync.dma_start(bias_sb[:], bias_pre)
            for c, e in BF_CHUNKS:
                nc.sync.dma_start(wb_all[:, c:e, :], wb_v[:, c:e, :])
                nc.sync.dma_start(xbs0[:, c:e, :], xb_v[:, c:e, 0:NT])
            if n_nt > 2:
                x8strips[2] = load_x8_strip(2)
            for j in (1, 2):
                if j < n_nt:
                    xbstrips[j] = load_xb_strip(j)
            # LoRA constants: first consumer is xa_block(0) at the end of
            # iteration j=1, so these can trail the j<=2 strips
            nc.sync.dma_start(a8_all[:], a8_v)
            nc.sync.dma_start(ab_all[:], ab_v)
            nc.sync.dma_start(mask_sb[:], mask_own)
            nc.sync.dma_start(b_cat[:], b_cat_t)

            # ---- pipelined compute: the fp8 DR half of token-tiles j and
            # j+1 runs ahead (8 PSUM banks); each iteration finishes tile j
            # with its bf16 half, then opens tile j+2's fp8 half
            alloc_groups(0)
            dr_ntile(0, x8strips[0], PAIR_CHUNKS)
            if n_nt > 1:
                alloc_groups(1)
                dr_ntile(1, x8strips[1], [(0, NPR)])
            for j in range(n_nt):
                bf_ntile(j, xbstrips[j], k_outer=(j == 0))
                # xa for tile j-1 runs one iteration late so the LoRA
                # constants are off the startup DMA critical path
                if 1 <= j <= n_ot:
                    xa_block(j - 1, x8strips[j - 1], xbstrips[j - 1])
                nxt = j + 2
                if nxt < n_nt:
                    if nxt not in x8strips:
                        x8strips[nxt] = load_x8_strip(nxt)
                    if nxt not in xbstrips:
                        xbstrips[nxt] = load_xb_strip(nxt)
                    alloc_groups(nxt)
                    dr_ntile(nxt, x8strips[nxt], [(0, NPR)])
                x8strips.pop(j - 2, None)
                xbstrips.pop(j - 1, None)
            while delta_jobs:
                emit_delta(len(delta_jobs))

    nc.finalize()
    return nc


def _get_nc(key):
    if key not in _NC_CACHE:
        _NC_CACHE[key] = build_nc(*key)
    return _NC_CACHE[key]


def make_in_maps(x, adapter_ids, weight, bias, A_buffer, B_buffer, n_cores=_N_CORES):
    """Host-side shard + layout + quantization prep. Returns (in_maps, shapes)."""
    bf16 = ml_dtypes.bfloat16
    e4 = ml_dtypes.float8_e4m3
    x = np.asarray(x, dtype=np.float32)
    adapter_ids = np.asarray(adapter_ids, dtype=np.int32)
    weight = np.asarray(weight, dtype=np.float32)
    bias = np.asarray(bias, dtype=np.float32)
    A_buffer = np.asarray(A_buffer, dtype=np.float32)
    B_buffer = np.asarray(B_buffer, dtype=np.float32)

    S, D_IN = x.shape
    D_OUT = weight.shape[0]
    L, R, _ = A_buffer.shape
    d_loc = D_OUT // n_cores
    s_own = S // n_cores
    LR = L * R
    K8 = _N8 * _P
    C = _SX * _SW
    C2 = _SX * _SA
    assert LR == _LR

    def q8(a, scale):
        return np.clip(a * scale, -240.0, 240.0).astype(e4)

    x8T = np.ascontiguousarray(q8(x[:, :K8], _SX).T)  # [K8, S] fp8
    xbT = np.ascontiguousarray(x[:, K8:].astype(bf16).T)  # [D_IN-K8, S] bf16
    Af = A_buffer.reshape(LR, D_IN)
    a8_t = np.ascontiguousarray(q8(Af[:, :K8], _SA).T)
    ab_t = np.ascontiguousarray((Af[:, K8:] * C2).astype(bf16).T)
    b_cat_t = np.ascontiguousarray(
        B_buffer.transpose(0, 2, 1).reshape(LR, D_OUT).astype(bf16)
    )
    maskT = (np.arange(LR)[:, None] // R == adapter_ids[None, :]).astype(bf16)

    in_maps = []
    for i in range(n_cores):
        osl = slice(i * d_loc, (i + 1) * d_loc)
        w8_t = np.ascontiguousarray(q8(weight[osl, :K8], _SW).T)  # [K8, d_loc]
        wb_t = np.ascontiguousarray((weight[osl, K8:] * C).astype(bf16).T)
        bias_pre = np.ascontiguousarray((bias[osl] * C).reshape(d_loc // _P, _P).T)
        # rotate the token axis so core i's own slab comes first
        sh = -i * s_own
        in_maps.append(
            {
                "x8T": np.ascontiguousarray(np.roll(x8T, sh, axis=1)) if i else x8T,
                "xbT": np.ascontiguousarray(np.roll(xbT, sh, axis=1)) if i else xbT,
                "w8_t": w8_t,
                "wb_t": wb_t,
                "a8_t": a8_t,
                "ab_t": ab_t,
                "b_cat_t": b_cat_t,
                "mask_own": np.ascontiguousarray(
                    maskT[:, i * s_own : (i + 1) * s_own]
                ),
                "bias_pre": bias_pre,
            }
        )
    return in_maps, (S, D_IN, D_OUT, d_loc, s_own)


def kernel(x, adapter_ids, weight, bias, A_buffer, B_buffer):
    global LAST_RESULTS
    _import_concourse()
    from concourse.bass_utils import run_bass_kernel_spmd

    in_maps, (S, D_IN, D_OUT, d_loc, s_own) = make_in_maps(
        x, adapter_ids, weight, bias, A_buffer, B_buffer
    )
    nc = _get_nc((D_IN, d_loc, S, s_own, D_OUT))
    LAST_RESULTS = run_bass_kernel_spmd(nc, in_maps, core_ids=list(range(_N_CORES)))
    res = LAST_RESULTS.results
    inv_c = np.float32(1.0) / (_SX * _SW)
    inv_c2 = np.float32(1.0) / (_SX * _SA)
    out = np.empty((S, D_OUT), dtype=np.float32)
    for i in range(_N_CORES):
        # un-rotate this core's token axis while scattering its base shard
        base = res[i]["out_t"].astype(np.float32) * inv_c
        if i:
            base = np.roll(base, i * s_own, axis=1)
        out[:, i * d_loc : (i + 1) * d_loc] = base.T
    for i in range(_N_CORES):
        out[i * s_own : (i + 1) * s_own, :] += (
            res[i]["delta_t"].T.astype(np.float32) * inv_c2
        )
    return out


# revision 31
# speedup vs baseline: 1.0044x; 1.0044x over previous
"""Trainium2 Bass kernel: ColumnParallelLinear + multi-adapter LoRA routing.

Computes out = x @ W^T + bias + B[aid[s]] @ (A[aid[s]] @ x[s]) for each token.

Distribution across 8 NeuronCores (one TRN2 chip):
  - base GEMM is tensor-parallel over d_out (sharding_hint): weight + bias
    sharded, each core emits out_base^T [512, S]
  - the LoRA delta is token-parallel: core i computes the delta for ITS
    1024-token slab across ALL d_out (A and B are tiny and replicated); the
    host adds the two partial results while unsharding
  - each core's token axis is ROTATED on the host so its own slab occupies
    the first two 512-token tiles; the xa matmuls then reuse the base
    x-strips already in SBUF; the host un-rotates while unsharding

Precision strategy (rel-err budget 2e-2, measured 1.95e-2 end to end):
  - the first N8=12 of 32 k-tiles run in fp8 e4m3 with perf_mode=DoubleRow:
    one pair-instruction contracts K=256 in the time of a single bf16 MM
    (measured 216ns back-to-back, the full 2x) -> 6 pair MMs replace 12
  - fp8 operands are pre-scaled on the host (x*16, w*1024); the remaining
    bf16 k-tiles use weights pre-scaled by C=16384 so both parts accumulate
    in the SAME PSUM group; outputs are stored C-scaled in bf16 and the
    host divides while unsharding (no extra on-chip work)
  - the LoRA-A projection uses the same k-split; B-matmuls stay bf16

Schedule:
  - startup: fp8 w + fp8 x-strip0 land first (1.5MB in a few large DMAs at
    ~400GB/s aggregate - single dma_starts stripe over all 16 DMA engines);
    j=0 runs "k-outer" so every arriving k-chunk immediately feeds all 4
    m-tile PSUM groups -> PE is busy ~2us after the DMA preamble
  - bias is added during base PSUM->SBUF eviction (per-partition scalar add)
  - the 64 B_cat delta matmuls are drip-fed 2-per-base-evict
  - outputs are stored as scaled bf16 (halves store traffic; the final
    stores are split into pieces so the kernel tail stays short)
"""

import os
import sys

import numpy as np

try:
    import ml_dtypes
except ImportError:  # pragma: no cover
    sys.path.insert(0, "/opt/trn_rl_repo")
    import ml_dtypes

_P = 128  # SBUF partitions / matmul tile edge
_NT = 512  # token tile (matmul moving free dim, one PSUM bank of fp32)
_LR = 128  # L * R = 8 * 16 adapter-rank rows
_N_CORES = 8
_N8 = 12  # k-tiles (of 32) computed in fp8 e4m3 DoubleRow
_SX = np.float32(16.0)  # x fp8 scale
_SW = np.float32(1024.0)  # w fp8 scale
_SA = np.float32(1024.0)  # A fp8 scale

_NC_CACHE = {}
LAST_RESULTS = None  # BassKernelResults of the most recent run (for test.py)


def _import_concourse():
    try:
        import concourse  # noqa: F401
    except ImportError:  # pragma: no cover
        for p in ("/opt/trn_rl_repo", "/root/.axon_site/_ro/trn_rl_repo"):
            if os.path.isdir(p) and p not in sys.path:
                sys.path.insert(0, p)


def build_nc(d_in: int, d_loc: int, s_tokens: int, s_own: int, d_out: int):
    """Build + finalize the per-core Bass kernel.

    d_loc: output features of this core's base shard
    s_own: tokens in this core's LoRA-delta slab (the FIRST s_own tokens of
           the core's rotated token order)
    d_out: full output width (the delta covers all of it)
    """
    _import_concourse()
    import concourse.tile as tile
    from concourse import bacc, mybir

    P, NT, LR, N8 = _P, _NT, _LR, _N8
    n_kt = d_in // P
    NB = n_kt - N8  # bf16 k-tiles
    NPR = N8 // 2  # fp8 DoubleRow pairs
    n_mt = d_loc // P
    n_nt = s_tokens // NT
    n_ot = s_own // NT  # own-slab token tiles
    n_dt = d_out // P  # delta feature tiles
    assert all(v % P == 0 for v in (d_in, d_loc, d_out)) and s_tokens % NT == 0
    assert s_own % NT == 0 and n_ot <= n_nt and N8 % 2 == 0

    nc = bacc.Bacc("TRN2", target_bir_lowering=False, debug=False)

    bf16 = mybir.dt.bfloat16
    f8 = mybir.dt.float8e4
    f32 = mybir.dt.float32
    DR = mybir.MatmulPerfMode.DoubleRow

    x8T = nc.dram_tensor("x8T", [N8 * P, s_tokens], f8, kind="ExternalInput").ap()
    xbT = nc.dram_tensor("xbT", [NB * P, s_tokens], bf16, kind="ExternalInput").ap()
    w8_t = nc.dram_tensor("w8_t", [N8 * P, d_loc], f8, kind="ExternalInput").ap()
    wb_t = nc.dram_tensor("wb_t", [NB * P, d_loc], bf16, kind="ExternalInput").ap()
    a8_t = nc.dram_tensor("a8_t", [N8 * P, LR], f8, kind="ExternalInput").ap()
    ab_t = nc.dram_tensor("ab_t", [NB * P, LR], bf16, kind="ExternalInput").ap()
    b_cat_t = nc.dram_tensor("b_cat_t", [LR, d_out], bf16, kind="ExternalInput").ap()
    mask_own = nc.dram_tensor("mask_own", [LR, s_own], bf16, kind="ExternalInput").ap()
    bias_pre = nc.dram_tensor("bias_pre", [P, n_mt], f32, kind="ExternalInput").ap()
    out_t = nc.dram_tensor("out_t", [d_loc, s_tokens], bf16, kind="ExternalOutput").ap()
    delta_t = nc.dram_tensor("delta_t", [d_out, s_own], bf16, kind="ExternalOutput").ap()

    # [kt*128 + p, n] -> [p, kt, n]
    x8_v = x8T.rearrange("(kt p) s -> p kt s", p=P)
    xb_v = xbT.rearrange("(kt p) s -> p kt s", p=P)
    w8_v = w8_t.rearrange("(kt p) m -> p kt m", p=P)
    wb_v = wb_t.rearrange("(kt p) m -> p kt m", p=P)
    a8_v = a8_t.rearrange("(kt p) m -> p kt m", p=P)
    ab_v = ab_t.rearrange("(kt p) m -> p kt m", p=P)

    # startup chunking: fp8 pairs first in fine chunks (PE starts ~2us after
    # the preamble), then bf16 weight/strip chunks interleaved 4 k-tiles at
    # a time — the head is chip-bandwidth-bound with all 8 cores loading at
    # once, so arrival order must match the j=0 k-outer consumption order
    PAIR_CHUNKS = [(0, 1), (1, 2), (2, 4), (4, NPR)]
    BF_CHUNKS = [(c, min(c + 4, NB)) for c in range(0, NB, 4)]

    with tile.TileContext(nc) as tc:
        with (
            tc.tile_pool(name="const", bufs=1) as const_pool,
            tc.tile_pool(name="xp", bufs=1) as x_pool,
            tc.tile_pool(name="outp", bufs=1) as out_pool,
            tc.tile_pool(name="psum", bufs=1, space="PSUM") as psum_pool,
        ):
            w8_all = const_pool.tile([P, N8, d_loc], f8)
            wb_all = const_pool.tile([P, NB, d_loc], bf16)
            b_cat = const_pool.tile([P, d_out], bf16)
            bias_sb = const_pool.tile([P, n_mt], f32)
            a8_all = const_pool.tile([P, N8, LR], f8)
            ab_all = const_pool.tile([P, NB, LR], bf16)
            xa_sb = const_pool.tile([P, s_own], bf16)
            mask_sb = const_pool.tile([P, s_own], bf16)

            # All PSUM tiles share one 8-bank ring so that during the DMA-
            # paced startup TWO token-tiles' worth of fp8 groups (8 banks)
            # can be open at once: the fp8 DoubleRow work of j and j+1 runs
            # one iteration ahead of j's bf16 work (software pipelining),
            # which keeps the PE busy while the 5MB bf16 block streams in.
            def ring_tile(name):
                return psum_pool.tile([P, NT], f32, tag="ring", bufs=8, name=name)

            # Deferred LoRA-delta jobs, drip-fed between base m-tiles.
            delta_jobs = []

            def emit_delta(k):
                for _ in range(k):
                    if not delta_jobs:
                        return
                    n, m = delta_jobs.pop(0)
                    dl_ps = ring_tile(f"dl_ps{n}_{m}")
                    nc.tensor.matmul(
                        dl_ps[:],
                        b_cat[:, m * P : (m + 1) * P],
                        xa_sb[:, n * NT : (n + 1) * NT],
                        start=True,
                        stop=True,
                    )
                    d_sb = out_pool.tile(
                        [P, NT], bf16, tag="d_sb", bufs=4, name=f"d_sb{n}_{m}"
                    )
                    nc.scalar.copy(d_sb[:], dl_ps[:])
                    nc.sync.dma_start(
                        delta_t[m * P : (m + 1) * P, n * NT : (n + 1) * NT], d_sb[:]
                    )

            def load_x8_strip(j):
                tok = slice(j * NT, (j + 1) * NT)
                x8s = x_pool.tile([P, N8, NT], f8, tag="x8s", bufs=4, name=f"x8s{j}")
                nc.sync.dma_start(x8s[:], x8_v[:, :, tok])
                return x8s

            def load_xb_strip(j):
                tok = slice(j * NT, (j + 1) * NT)
                xbs = x_pool.tile([P, NB, NT], bf16, tag="xbs", bufs=4, name=f"xbs{j}")
                for c, e in ((0, NB // 2), (NB // 2, NB)):
                    nc.sync.dma_start(xbs[:, c:e, :], xb_v[:, c:e, tok])
                return xbs

            def evict_base(j, m, ps):
                tok0 = j * NT
                o_sb = out_pool.tile(
                    [P, NT], bf16, tag="o_sb", bufs=6, name=f"o_sb{j}_{m}"
                )
                nc.vector.tensor_scalar_add(
                    out=o_sb[:], in0=ps[:], scalar1=bias_sb[:, m : m + 1]
                )
                nc.sync.dma_start(
                    out_t[m * P : (m + 1) * P, tok0 : tok0 + NT], o_sb[:]
                )
                emit_delta(2)

            def base_group_mms(ps, m, x8s, xbs, pr_range, kb_range):
                for pr in pr_range:
                    nc.tensor.matmul(
                        ps[:],
                        w8_all[:, 2 * pr : 2 * pr + 2, m * P : (m + 1) * P],
                        x8s[:, 2 * pr : 2 * pr + 2, :],
                        start=(pr == 0),
                        stop=False,
                        perf_mode=DR,
                    )
                for kt in kb_range:
                    nc.tensor.matmul(
                        ps[:],
                        wb_all[:, kt, m * P : (m + 1) * P],
                        xbs[:, kt, :],
                        start=False,
                        stop=(kt == NB - 1),
                    )

            pss = {}  # j -> list of 4 open PSUM groups (fp8 part done)

            def alloc_groups(j):
                pss[j] = [ring_tile(f"ps{j}_{m}") for m in range(n_mt)]

            def dr_ntile(j, x8s, pair_chunks):
                # the fp8 DoubleRow half of token-tile j, k-outer across all
                # m-tiles per pair chunk
                for c, e in pair_chunks:
                    for m in range(n_mt):
                        base_group_mms(pss[j][m], m, x8s, None, range(c, e), ())

            def bf_half_group(j, m, xbs, h):
                # last group of the kernel: run the bf16 part as two N=256
                # halves so the first half's evict+store overlaps the second
                # half's matmuls and the kernel tail shrinks
                ps = pss[j][m]
                sl = slice(h * (NT // 2), (h + 1) * (NT // 2))
                for kt in range(NB):
                    nc.tensor.matmul(
                        ps[:, sl],
                        wb_all[:, kt, m * P : (m + 1) * P],
                        xbs[:, kt, sl],
                        start=False,
                        stop=(kt == NB - 1),
                        skip_group_check=True,
                    )
                o_sb = out_pool.tile(
                    [P, NT // 2], bf16, tag="o_half", bufs=2, name=f"oh{j}_{m}_{h}"
                )
                nc.vector.tensor_scalar_add(
                    out=o_sb[:], in0=ps[:, sl], scalar1=bias_sb[:, m : m + 1]
                )
                nc.sync.dma_start(
                    out_t[
                        m * P : (m + 1) * P,
                        j * NT + h * (NT // 2) : j * NT + (h + 1) * (NT // 2),
                    ],
                    o_sb[:],
                )

            def bf_ntile(j, xbs, k_outer=False):
                # the bf16 half; for j=0 consume 4-k-tile chunks across all
                # m-tiles in DMA arrival order
                if not k_outer:
                    for m in range(n_mt):
                        if j == n_nt - 1 and m == n_mt - 1:
                            bf_half_group(j, m, xbs, 0)
                            bf_half_group(j, m, xbs, 1)
                            emit_delta(2)
                        else:
                            base_group_mms(pss[j][m], m, None, xbs, (), range(NB))
                            evict_base(j, m, pss[j][m])
                    return
                for c, e in BF_CHUNKS:
                    for m in range(n_mt):
                        base_group_mms(pss[j][m], m, None, xbs, (), range(c, e))
                for m in range(n_mt):
                    evict_base(j, m, pss[j][m])

            def xa_block(n, x8s, xbs):
                # xa = A_all @ x^T for own-slab tile n, masked per-token;
                # queues that tile's 32 B_cat delta matmuls
                xa_ps = ring_tile(f"xa_ps{n}")
                for pr in range(NPR):
                    nc.tensor.matmul(
                        xa_ps[:],
                        a8_all[:, 2 * pr : 2 * pr + 2, :],
                        x8s[:, 2 * pr : 2 * pr + 2, :],
                        start=(pr == 0),
                        stop=False,
                        perf_mode=DR,
                    )
                for kt in range(NB):
                    nc.tensor.matmul(
                        xa_ps[:],
                        ab_all[:, kt, :],
                        xbs[:, kt, :],
                        start=False,
                        stop=(kt == NB - 1),
                    )
                nc.vector.tensor_mul(
                    out=xa_sb[:, n * NT : (n + 1) * NT],
                    in0=xa_ps[:],
                    in1=mask_sb[:, n * NT : (n + 1) * NT],
                )
                delta_jobs.extend((n, m) for m in range(n_dt))

            # ---- startup DMA: all on the Sync queue (transfers issued from
            # the Scalar/GpSimd queues measured slower), in consumption
            # order: fp8 pairs for j=0, bias, the j=1 fp8 strip (feeds the
            # pipelined-ahead DR work), interleaved bf16 w/x chunks, the
            # rest of strips j=1,2, LoRA constants
            x8s0 = x_pool.tile([P, N8, NT], f8, tag="x8s", bufs=4, name="x8s_first")
            xbs0 = x_pool.tile([P, NB, NT], bf16, tag="xbs", bufs=4, name="xbs_first")
            x8strips = {0: x8s0}
            xbstrips = {0: xbs0}
            for ci, (c, e) in enumerate(PAIR_CHUNKS):
                nc.sync.dma_start(
                    w8_all[:, 2 * c : 2 * e, :], w8_v[:, 2 * c : 2 * e, :]
                )
                nc.sync.dma_start(
                    x8s0[:, 2 * c : 2 * e, :], x8_v[:, 2 * c : 2 * e, 0:NT]
                )
                if ci == 0 and n_nt > 1:
                    # j=1's fp8 strip feeds the pipelined-ahead DR work; it
                    # must land right behind j=0's fp8 block
                    x8strips[1] = load_x8_strip(1)
            nc.sync.dma_start(bias_sb[:], bias_pre)
            for c, e in BF_CHUNKS:
                nc.sync.dma_start(wb_all[:, c:e, :], wb_v[:, c:e, :])
                nc.sync.dma_start(xbs0[:, c:e, :], xb_v[:, c:e, 0:NT])
            if n_nt > 2:
                x8strips[2] = load_x8_strip(2)
            for j in (1, 2):
                if j < n_nt:
                    xbstrips[j] = load_xb_strip(j)
            # LoRA constants: first consumer is xa_block(0) at the end of
            # iteration j=1, so these can trail the j<=2 strips
            nc.sync.dma_start(a8_all[:], a8_v)
            nc.sync.dma_start(ab_all[:], ab_v)
            nc.sync.dma_start(mask_sb[:], mask_own)
            nc.sync.dma_start(b_cat[:], b_cat_t)

            # ---- pipelined compute: the fp8 DR half of token-tiles j and
            # j+1 runs ahead (8 PSUM banks); each iteration finishes tile j
            # with its bf16 half, then opens tile j+2's fp8 half
            alloc_groups(0)
            dr_ntile(0, x8strips[0], PAIR_CHUNKS)
            if n_nt > 1:
                alloc_groups(1)
                dr_ntile(1, x8strips[1], [(0, NPR)])
            for j in range(n_nt):
                bf_ntile(j, xbstrips[j], k_outer=(j == 0))
                # xa for tile j-1 runs one iteration late so the LoRA
                # constants are off the startup DMA critical path
                if 1 <= j <= n_ot:
                    xa_block(j - 1, x8strips[j - 1], xbstrips[j - 1])
                nxt = j + 2
                if nxt < n_nt:
                    if nxt not in x8strips:
                        x8strips[nxt] = load_x8_strip(nxt)
                    if nxt not in xbstrips:
                        xbstrips[nxt] = load_xb_strip(nxt)
                    alloc_groups(nxt)
                    dr_ntile(nxt, x8strips[nxt], [(0, NPR)])
                x8strips.pop(j - 2, None)
                xbstrips.pop(j - 1, None)
            while delta_jobs:
                emit_delta(len(delta_jobs))

    nc.finalize()
    return nc


def _get_nc(key):
    if key not in _NC_CACHE:
        _NC_CACHE[key] = build_nc(*key)
    return _NC_CACHE[key]


def make_in_maps(x, adapter_ids, weight, bias, A_buffer, B_buffer, n_cores=_N_CORES):
    """Host-side shard + layout + quantization prep. Returns (in_maps, shapes)."""
    bf16 = ml_dtypes.bfloat16
    e4 = ml_dtypes.float8_e4m3
    x = np.asarray(x, dtype=np.float32)
    adapter_ids = np.asarray(adapter_ids, dtype=np.int32)
    weight = np.asarray(weight, dtype=np.float32)
    bias = np.asarray(bias, dtype=np.float32)
    A_buffer = np.asarray(A_buffer, dtype=np.float32)
    B_buffer = np.asarray(B_buffer, dtype=np.float32)

    S, D_IN = x.shape
    D_OUT = weight.shape[0]
    L, R, _ = A_buffer.shape
    d_loc = D_OUT // n_cores
    s_own = S // n_cores
    LR = L * R
    K8 = _N8 * _P
    C = _SX * _SW
    C2 = _SX * _SA
    assert LR == _LR

    def q8(a, scale):
        return np.clip(a * scale, -240.0, 240.0).astype(e4)

    x8T = np.ascontiguousarray(q8(x[:, :K8], _SX).T)  # [K8, S] fp8
    xbT = np.ascontiguousarray(x[:, K8:].astype(bf16).T)  # [D_IN-K8, S] bf16
    Af = A_buffer.reshape(LR, D_IN)
    a8_t = np.ascontiguousarray(q8(Af[:, :K8], _SA).T)
    ab_t = np.ascontiguousarray((Af[:, K8:] * C2).astype(bf16).T)
    b_cat_t = np.ascontiguousarray(
        B_buffer.transpose(0, 2, 1).reshape(LR, D_OUT).astype(bf16)
    )
    maskT = (np.arange(LR)[:, None] // R == adapter_ids[None, :]).astype(bf16)

    in_maps = []
    for i in range(n_cores):
        osl = slice(i * d_loc, (i + 1) * d_loc)
        w8_t = np.ascontiguousarray(q8(weight[osl, :K8], _SW).T)  # [K8, d_loc]
        wb_t = np.ascontiguousarray((weight[osl, K8:] * C).astype(bf16).T)
        bias_pre = np.ascontiguousarray((bias[osl] * C).reshape(d_loc // _P, _P).T)
        # rotate the token axis so core i's own slab comes first
        sh = -i * s_own
        in_maps.append(
            {
                "x8T": np.ascontiguousarray(np.roll(x8T, sh, axis=1)) if i else x8T,
                "xbT": np.ascontiguousarray(np.roll(xbT, sh, axis=1)) if i else xbT,
                "w8_t": w8_t,
                "wb_t": wb_t,
                "a8_t": a8_t,
                "ab_t": ab_t,
                "b_cat_t": b_cat_t,
                "mask_own": np.ascontiguousarray(
                    maskT[:, i * s_own : (i + 1) * s_own]
                ),
                "bias_pre": bias_pre,
            }
        )
    return in_maps, (S, D_IN, D_OUT, d_loc, s_own)


def kernel(x, adapter_ids, weight, bias, A_buffer, B_buffer):
    global LAST_RESULTS
    _import_concourse()
    from concourse.bass_utils import run_bass_kernel_spmd

    in_maps, (S, D_IN, D_OUT, d_loc, s_own) = make_in_maps(
        x, adapter_ids, weight, bias, A_buffer, B_buffer
    )
    nc = _get_nc((D_IN, d_loc, S, s_own, D_OUT))
    LAST_RESULTS = run_bass_kernel_spmd(nc, in_maps, core_ids=list(range(_N_CORES)))
    res = LAST_RESULTS.results
    inv_c = np.float32(1.0) / (_SX * _SW)
    inv_c2 = np.float32(1.0) / (_SX * _SA)
    out = np.empty((S, D_OUT), dtype=np.float32)
    for i in range(_N_CORES):
        # un-rotate this core's token axis while scattering its base shard
        base = res[i]["out_t"].astype(np.float32) * inv_c
        if i:
            base = np.roll(base, i * s_own, axis=1)
        out[:, i * d_loc : (i + 1) * d_loc] = base.T
    for i in range(_N_CORES):
        out[i * s_own : (i + 1) * s_own, :] += (
            res[i]["delta_t"].T.astype(np.float32) * inv_c2
        )
    return out


# revision 32
# speedup vs baseline: 1.0123x; 1.0079x over previous
"""Trainium2 Bass kernel: ColumnParallelLinear + multi-adapter LoRA routing.

Computes out = x @ W^T + bias + B[aid[s]] @ (A[aid[s]] @ x[s]) for each token.

Distribution across 8 NeuronCores (one TRN2 chip):
  - base GEMM is tensor-parallel over d_out (sharding_hint): weight + bias
    sharded, each core emits out_base^T [512, S]
  - the LoRA delta is token-parallel: core i computes the delta for ITS
    1024-token slab across ALL d_out (A and B are tiny and replicated); the
    host adds the two partial results while unsharding
  - each core's token axis is ROTATED on the host so its own slab occupies
    the first two 512-token tiles; the xa matmuls then reuse the base
    x-strips already in SBUF; the host un-rotates while unsharding

Precision strategy (rel-err budget 2e-2, measured 1.95e-2 end to end):
  - the first N8=12 of 32 k-tiles run in fp8 e4m3 with perf_mode=DoubleRow:
    one pair-instruction contracts K=256 in the time of a single bf16 MM
    (measured 216ns back-to-back, the full 2x) -> 6 pair MMs replace 12
  - fp8 operands are pre-scaled on the host (x*16, w*1024); the remaining
    bf16 k-tiles use weights pre-scaled by C=16384 so both parts accumulate
    in the SAME PSUM group; outputs are stored C-scaled in bf16 and the
    host divides while unsharding (no extra on-chip work)
  - the LoRA-A projection uses the same k-split; B-matmuls stay bf16

Schedule:
  - startup: fp8 w + fp8 x-strip0 land first (1.5MB in a few large DMAs at
    ~400GB/s aggregate - single dma_starts stripe over all 16 DMA engines);
    j=0 runs "k-outer" so every arriving k-chunk immediately feeds all 4
    m-tile PSUM groups -> PE is busy ~2us after the DMA preamble
  - bias is added during base PSUM->SBUF eviction (per-partition scalar add)
  - the 64 B_cat delta matmuls are drip-fed 2-per-base-evict
  - outputs are stored as scaled bf16 (halves store traffic; the final
    stores are split into pieces so the kernel tail stays short)
"""

import os
import sys

import numpy as np

try:
    import ml_dtypes
except ImportError:  # pragma: no cover
    sys.path.insert(0, "/opt/trn_rl_repo")
    import ml_dtypes

_P = 128  # SBUF partitions / matmul tile edge
_NT = 512  # token tile (matmul moving free dim, one PSUM bank of fp32)
_LR = 128  # L * R = 8 * 16 adapter-rank rows
_N_CORES = 8
_N8 = 12  # k-tiles (of 32) computed in fp8 e4m3 DoubleRow
_SX = np.float32(16.0)  # x fp8 scale
_SW = np.float32(1024.0)  # w fp8 scale
_SA = np.float32(1024.0)  # A fp8 scale

_NC_CACHE = {}
LAST_RESULTS = None  # BassKernelResults of the most recent run (for test.py)


def _import_concourse():
    try:
        import concourse  # noqa: F401
    except ImportError:  # pragma: no cover
        for p in ("/opt/trn_rl_repo", "/root/.axon_site/_ro/trn_rl_repo"):
            if os.path.isdir(p) and p not in sys.path:
                sys.path.insert(0, p)


def build_nc(d_in: int, d_loc: int, s_tokens: int, s_own: int, d_out: int):
    """Build + finalize the per-core Bass kernel.

    d_loc: output features of this core's base shard
    s_own: tokens in this core's LoRA-delta slab (the FIRST s_own tokens of
           the core's rotated token order)
    d_out: full output width (the delta covers all of it)
    """
    _import_concourse()
    import concourse.tile as tile
    from concourse import bacc, mybir

    P, NT, LR, N8 = _P, _NT, _LR, _N8
    n_kt = d_in // P
    NB = n_kt - N8  # bf16 k-tiles
    NPR = N8 // 2  # fp8 DoubleRow pairs
    n_mt = d_loc // P
    n_nt = s_tokens // NT
    n_ot = s_own // NT  # own-slab token tiles
    n_dt = d_out // P  # delta feature tiles
    assert all(v % P == 0 for v in (d_in, d_loc, d_out)) and s_tokens % NT == 0
    assert s_own % NT == 0 and n_ot <= n_nt and N8 % 2 == 0

    nc = bacc.Bacc("TRN2", target_bir_lowering=False, debug=False)

    bf16 = mybir.dt.bfloat16
    f8 = mybir.dt.float8e4
    f32 = mybir.dt.float32
    DR = mybir.MatmulPerfMode.DoubleRow

    x8T = nc.dram_tensor("x8T", [N8 * P, s_tokens], f8, kind="ExternalInput").ap()
    xbT = nc.dram_tensor("xbT", [NB * P, s_tokens], bf16, kind="ExternalInput").ap()
    w8_t = nc.dram_tensor("w8_t", [N8 * P, d_loc], f8, kind="ExternalInput").ap()
    wb_t = nc.dram_tensor("wb_t", [NB * P, d_loc], bf16, kind="ExternalInput").ap()
    a8_t = nc.dram_tensor("a8_t", [N8 * P, LR], f8, kind="ExternalInput").ap()
    ab_t = nc.dram_tensor("ab_t", [NB * P, LR], bf16, kind="ExternalInput").ap()
    b_cat_t = nc.dram_tensor("b_cat_t", [LR, d_out], bf16, kind="ExternalInput").ap()
    mask_own = nc.dram_tensor("mask_own", [LR, s_own], bf16, kind="ExternalInput").ap()
    bias_pre = nc.dram_tensor("bias_pre", [P, n_mt], f32, kind="ExternalInput").ap()
    out_t = nc.dram_tensor("out_t", [d_loc, s_tokens], bf16, kind="ExternalOutput").ap()
    delta_t = nc.dram_tensor("delta_t", [d_out, s_own], bf16, kind="ExternalOutput").ap()

    # [kt*128 + p, n] -> [p, kt, n]
    x8_v = x8T.rearrange("(kt p) s -> p kt s", p=P)
    xb_v = xbT.rearrange("(kt p) s -> p kt s", p=P)
    w8_v = w8_t.rearrange("(kt p) m -> p kt m", p=P)
    wb_v = wb_t.rearrange("(kt p) m -> p kt m", p=P)
    a8_v = a8_t.rearrange("(kt p) m -> p kt m", p=P)
    ab_v = ab_t.rearrange("(kt p) m -> p kt m", p=P)

    # startup chunking: fp8 pairs first in fine chunks (PE starts ~2us after
    # the preamble), then bf16 weight/strip chunks interleaved 4 k-tiles at
    # a time — the head is chip-bandwidth-bound with all 8 cores loading at
    # once, so arrival order must match the j=0 k-outer consumption order
    PAIR_CHUNKS = [(0, 1), (1, 2), (2, 4), (4, NPR)]
    BF_CHUNKS = [(c, min(c + 4, NB)) for c in range(0, NB, 4)]

    with tile.TileContext(nc) as tc:
        with (
            tc.tile_pool(name="const", bufs=1) as const_pool,
            tc.tile_pool(name="xp", bufs=1) as x_pool,
            tc.tile_pool(name="outp", bufs=1) as out_pool,
            tc.tile_pool(name="psum", bufs=1, space="PSUM") as psum_pool,
        ):
            w8_all = const_pool.tile([P, N8, d_loc], f8)
            wb_all = const_pool.tile([P, NB, d_loc], bf16)
            b_cat = const_pool.tile([P, d_out], bf16)
            bias_sb = const_pool.tile([P, n_mt], f32)
            a8_all = const_pool.tile([P, N8, LR], f8)
            ab_all = const_pool.tile([P, NB, LR], bf16)
            xa_sb = const_pool.tile([P, s_own], bf16)
            mask_sb = const_pool.tile([P, s_own], bf16)

            # All PSUM tiles share one 8-bank ring so that during the DMA-
            # paced startup TWO token-tiles' worth of fp8 groups (8 banks)
            # can be open at once: the fp8 DoubleRow work of j and j+1 runs
            # one iteration ahead of j's bf16 work (software pipelining),
            # which keeps the PE busy while the 5MB bf16 block streams in.
            def ring_tile(name):
                return psum_pool.tile([P, NT], f32, tag="ring", bufs=8, name=name)

            # Deferred LoRA-delta jobs, drip-fed between base m-tiles.
            delta_jobs = []

            def emit_delta(k):
                for _ in range(k):
                    if not delta_jobs:
                        return
                    n, m = delta_jobs.pop(0)
                    dl_ps = ring_tile(f"dl_ps{n}_{m}")
                    nc.tensor.matmul(
                        dl_ps[:],
                        b_cat[:, m * P : (m + 1) * P],
                        xa_sb[:, n * NT : (n + 1) * NT],
                        start=True,
                        stop=True,
                    )
                    d_sb = out_pool.tile(
                        [P, NT], bf16, tag="d_sb", bufs=4, name=f"d_sb{n}_{m}"
                    )
                    nc.scalar.copy(d_sb[:], dl_ps[:])
                    nc.sync.dma_start(
                        delta_t[m * P : (m + 1) * P, n * NT : (n + 1) * NT], d_sb[:]
                    )

            def load_x8_strip(j):
                tok = slice(j * NT, (j + 1) * NT)
                x8s = x_pool.tile([P, N8, NT], f8, tag="x8s", bufs=4, name=f"x8s{j}")
                nc.sync.dma_start(x8s[:], x8_v[:, :, tok])
                return x8s

            def load_xb_strip(j):
                tok = slice(j * NT, (j + 1) * NT)
                xbs = x_pool.tile([P, NB, NT], bf16, tag="xbs", bufs=4, name=f"xbs{j}")
                for c, e in ((0, NB // 2), (NB // 2, NB)):
                    nc.sync.dma_start(xbs[:, c:e, :], xb_v[:, c:e, tok])
                return xbs

            def evict_base(j, m, ps):
                tok0 = j * NT
                o_sb = out_pool.tile(
                    [P, NT], bf16, tag="o_sb", bufs=6, name=f"o_sb{j}_{m}"
                )
                nc.vector.tensor_scalar_add(
                    out=o_sb[:], in0=ps[:], scalar1=bias_sb[:, m : m + 1]
                )
                nc.sync.dma_start(
                    out_t[m * P : (m + 1) * P, tok0 : tok0 + NT], o_sb[:]
                )
                emit_delta(2)

            def base_group_mms(ps, m, x8s, xbs, pr_range, kb_range):
                for pr in pr_range:
                    nc.tensor.matmul(
                        ps[:],
                        w8_all[:, 2 * pr : 2 * pr + 2, m * P : (m + 1) * P],
                        x8s[:, 2 * pr : 2 * pr + 2, :],
                        start=(pr == 0),
                        stop=False,
                        perf_mode=DR,
                    )
                for kt in kb_range:
                    nc.tensor.matmul(
                        ps[:],
                        wb_all[:, kt, m * P : (m + 1) * P],
                        xbs[:, kt, :],
                        start=False,
                        stop=(kt == NB - 1),
                    )

            pss = {}  # j -> list of 4 open PSUM groups (fp8 part done)

            def alloc_groups(j):
                pss[j] = [ring_tile(f"ps{j}_{m}") for m in range(n_mt)]

            def dr_ntile(j, x8s, pair_chunks):
                # the fp8 DoubleRow half of token-tile j, k-outer across all
                # m-tiles per pair chunk
                for c, e in pair_chunks:
                    for m in range(n_mt):
                        base_group_mms(pss[j][m], m, x8s, None, range(c, e), ())

            def bf_half_group(j, m, xbs, h):
                # last group of the kernel: run the bf16 part as two N=256
                # halves so the first half's evict+store overlaps the second
                # half's matmuls and the kernel tail shrinks
                ps = pss[j][m]
                sl = slice(h * (NT // 2), (h + 1) * (NT // 2))
                for kt in range(NB):
                    nc.tensor.matmul(
                        ps[:, sl],
                        wb_all[:, kt, m * P : (m + 1) * P],
                        xbs[:, kt, sl],
                        start=False,
                        stop=(kt == NB - 1),
                        skip_group_check=True,
                    )
                o_sb = out_pool.tile(
                    [P, NT // 2], bf16, tag="o_half", bufs=2, name=f"oh{j}_{m}_{h}"
                )
                nc.vector.tensor_scalar_add(
                    out=o_sb[:], in0=ps[:, sl], scalar1=bias_sb[:, m : m + 1]
                )
                nc.sync.dma_start(
                    out_t[
                        m * P : (m + 1) * P,
                        j * NT + h * (NT // 2) : j * NT + (h + 1) * (NT // 2),
                    ],
                    o_sb[:],
                )

            def bf_ntile(j, xbs, k_outer=False):
                # the bf16 half; for j=0 consume 4-k-tile chunks across all
                # m-tiles in DMA arrival order
                if not k_outer:
                    for m in range(n_mt):
                        if j == n_nt - 1 and m == n_mt - 1:
                            bf_half_group(j, m, xbs, 0)
                            bf_half_group(j, m, xbs, 1)
                            emit_delta(2)
                        else:
                            base_group_mms(pss[j][m], m, None, xbs, (), range(NB))
                            evict_base(j, m, pss[j][m])
                    return
                for c, e in BF_CHUNKS:
                    for m in range(n_mt):
                        base_group_mms(pss[j][m], m, None, xbs, (), range(c, e))
                for m in range(n_mt):
                    evict_base(j, m, pss[j][m])

            def xa_block(n, x8s, xbs):
                # xa = A_all @ x^T for own-slab tile n, masked per-token;
                # queues that tile's 32 B_cat delta matmuls
                xa_ps = ring_tile(f"xa_ps{n}")
                for pr in range(NPR):
                    nc.tensor.matmul(
                        xa_ps[:],
                        a8_all[:, 2 * pr : 2 * pr + 2, :],
                        x8s[:, 2 * pr : 2 * pr + 2, :],
                        start=(pr == 0),
                        stop=False,
                        perf_mode=DR,
                    )
                for kt in range(NB):
                    nc.tensor.matmul(
                        xa_ps[:],
                        ab_all[:, kt, :],
                        xbs[:, kt, :],
                        start=False,
                        stop=(kt == NB - 1),
                    )
                nc.vector.tensor_mul(
                    out=xa_sb[:, n * NT : (n + 1) * NT],
                    in0=xa_ps[:],
                    in1=mask_sb[:, n * NT : (n + 1) * NT],
                )
                delta_jobs.extend((n, m) for m in range(n_dt))

            # ---- startup DMA: all on the Sync queue (transfers issued from
            # the Scalar/GpSimd queues measured slower), in consumption
            # order: fp8 pairs for j=0, bias, the j=1 fp8 strip (feeds the
            # pipelined-ahead DR work), interleaved bf16 w/x chunks, the
            # rest of strips j=1,2, LoRA constants
            x8s0 = x_pool.tile([P, N8, NT], f8, tag="x8s", bufs=4, name="x8s_first")
            xbs0 = x_pool.tile([P, NB, NT], bf16, tag="xbs", bufs=4, name="xbs_first")
            x8strips = {0: x8s0}
            xbstrips = {0: xbs0}
            if n_nt > 1:
                x8s1 = x_pool.tile([P, N8, NT], f8, tag="x8s", bufs=4, name="x8s1")
                x8strips[1] = x8s1
            for ci, (c, e) in enumerate(PAIR_CHUNKS):
                nc.sync.dma_start(
                    w8_all[:, 2 * c : 2 * e, :], w8_v[:, 2 * c : 2 * e, :]
                )
                nc.sync.dma_start(
                    x8s0[:, 2 * c : 2 * e, :], x8_v[:, 2 * c : 2 * e, 0:NT]
                )
                # j=1's fp8 strip feeds the pipelined-ahead DR work; its two
                # halves land inside j=0's fp8 block without stalling it
                if ci in (1, 2) and n_nt > 1:
                    h = ci - 1
                    nc.sync.dma_start(
                        x8s1[:, h * (N8 // 2) : (h + 1) * (N8 // 2), :],
                        x8_v[:, h * (N8 // 2) : (h + 1) * (N8 // 2), NT : 2 * NT],
                    )
            nc.sync.dma_start(bias_sb[:], bias_pre)
            for c, e in BF_CHUNKS:
                nc.sync.dma_start(wb_all[:, c:e, :], wb_v[:, c:e, :])
                nc.sync.dma_start(xbs0[:, c:e, :], xb_v[:, c:e, 0:NT])
            if n_nt > 2:
                x8strips[2] = load_x8_strip(2)
            for j in (1, 2):
                if j < n_nt:
                    xbstrips[j] = load_xb_strip(j)
            # LoRA constants: first consumer is xa_block(0) at the end of
            # iteration j=1, so these can trail the j<=2 strips
            nc.sync.dma_start(a8_all[:], a8_v)
            nc.sync.dma_start(ab_all[:], ab_v)
            nc.sync.dma_start(mask_sb[:], mask_own)
            nc.sync.dma_start(b_cat[:], b_cat_t)

            # ---- pipelined compute: the fp8 DR half of token-tiles j and
            # j+1 runs ahead (8 PSUM banks); each iteration finishes tile j
            # with its bf16 half, then opens tile j+2's fp8 half
            alloc_groups(0)
            dr_ntile(0, x8strips[0], PAIR_CHUNKS)
            if n_nt > 1:
                alloc_groups(1)
                dr_ntile(1, x8strips[1], [(0, NPR)])
            for j in range(n_nt):
                bf_ntile(j, xbstrips[j], k_outer=(j == 0))
                # xa for tile j-1 runs one iteration late so the LoRA
                # constants are off the startup DMA critical path
                if 1 <= j <= n_ot:
                    xa_block(j - 1, x8strips[j - 1], xbstrips[j - 1])
                nxt = j + 2
                if nxt < n_nt:
                    if nxt not in x8strips:
                        x8strips[nxt] = load_x8_strip(nxt)
                    if nxt not in xbstrips:
                        xbstrips[nxt] = load_xb_strip(nxt)
                    alloc_groups(nxt)
                    dr_ntile(nxt, x8strips[nxt], [(0, NPR)])
                x8strips.pop(j - 2, None)
                xbstrips.pop(j - 1, None)
            while delta_jobs:
                emit_delta(len(delta_jobs))

    nc.finalize()
    return nc


def _get_nc(key):
    if key not in _NC_CACHE:
        _NC_CACHE[key] = build_nc(*key)
    return _NC_CACHE[key]


def make_in_maps(x, adapter_ids, weight, bias, A_buffer, B_buffer, n_cores=_N_CORES):
    """Host-side shard + layout + quantization prep. Returns (in_maps, shapes)."""
    bf16 = ml_dtypes.bfloat16
    e4 = ml_dtypes.float8_e4m3
    x = np.asarray(x, dtype=np.float32)
    adapter_ids = np.asarray(adapter_ids, dtype=np.int32)
    weight = np.asarray(weight, dtype=np.float32)
    bias = np.asarray(bias, dtype=np.float32)
    A_buffer = np.asarray(A_buffer, dtype=np.float32)
    B_buffer = np.asarray(B_buffer, dtype=np.float32)

    S, D_IN = x.shape
    D_OUT = weight.shape[0]
    L, R, _ = A_buffer.shape
    d_loc = D_OUT // n_cores
    s_own = S // n_cores
    LR = L * R
    K8 = _N8 * _P
    C = _SX * _SW
    C2 = _SX * _SA
    assert LR == _LR

    def q8(a, scale):
        return np.clip(a * scale, -240.0, 240.0).astype(e4)

    x8T = np.ascontiguousarray(q8(x[:, :K8], _SX).T)  # [K8, S] fp8
    xbT = np.ascontiguousarray(x[:, K8:].astype(bf16).T)  # [D_IN-K8, S] bf16
    Af = A_buffer.reshape(LR, D_IN)
    a8_t = np.ascontiguousarray(q8(Af[:, :K8], _SA).T)
    ab_t = np.ascontiguousarray((Af[:, K8:] * C2).astype(bf16).T)
    b_cat_t = np.ascontiguousarray(
        B_buffer.transpose(0, 2, 1).reshape(LR, D_OUT).astype(bf16)
    )
    maskT = (np.arange(LR)[:, None] // R == adapter_ids[None, :]).astype(bf16)

    in_maps = []
    for i in range(n_cores):
        osl = slice(i * d_loc, (i + 1) * d_loc)
        w8_t = np.ascontiguousarray(q8(weight[osl, :K8], _SW).T)  # [K8, d_loc]
        wb_t = np.ascontiguousarray((weight[osl, K8:] * C).astype(bf16).T)
        bias_pre = np.ascontiguousarray((bias[osl] * C).reshape(d_loc // _P, _P).T)
        # rotate the token axis so core i's own slab comes first
        sh = -i * s_own
        in_maps.append(
            {
                "x8T": np.ascontiguousarray(np.roll(x8T, sh, axis=1)) if i else x8T,
                "xbT": np.ascontiguousarray(np.roll(xbT, sh, axis=1)) if i else xbT,
                "w8_t": w8_t,
                "wb_t": wb_t,
                "a8_t": a8_t,
                "ab_t": ab_t,
                "b_cat_t": b_cat_t,
                "mask_own": np.ascontiguousarray(
                    maskT[:, i * s_own : (i + 1) * s_own]
                ),
                "bias_pre": bias_pre,
            }
        )
    return in_maps, (S, D_IN, D_OUT, d_loc, s_own)


def kernel(x, adapter_ids, weight, bias, A_buffer, B_buffer):
    global LAST_RESULTS
    _import_concourse()
    from concourse.bass_utils import run_bass_kernel_spmd

    in_maps, (S, D_IN, D_OUT, d_loc, s_own) = make_in_maps(
        x, adapter_ids, weight, bias, A_buffer, B_buffer
    )
    nc = _get_nc((D_IN, d_loc, S, s_own, D_OUT))
    LAST_RESULTS = run_bass_kernel_spmd(nc, in_maps, core_ids=list(range(_N_CORES)))
    res = LAST_RESULTS.results
    inv_c = np.float32(1.0) / (_SX * _SW)
    inv_c2 = np.float32(1.0) / (_SX * _SA)
    out = np.empty((S, D_OUT), dtype=np.float32)
    for i in range(_N_CORES):
        # un-rotate this core's token axis while scattering its base shard
        base = res[i]["out_t"].astype(np.float32) * inv_c
        if i:
            base = np.roll(base, i * s_own, axis=1)
        out[:, i * d_loc : (i + 1) * d_loc] = base.T
    for i in range(_N_CORES):
        out[i * s_own : (i + 1) * s_own, :] += (
            res[i]["delta_t"].T.astype(np.float32) * inv_c2
        )
    return out
